# revision 3
# baseline (speedup 1.0000x reference)
"""DenseToSparse kernel for Trainium2 (8 NeuronCores, batch-parallel).

Reference computation (per full input x [32, 256, 64, 64] fp32):
  feats = x.transpose(0,2,3,1).reshape(-1, 256)       # [131072, 256]
  active = |feats|.sum(axis=1) > 0                     # site mask
  out[j] = feats[sorted_active_sites[j]] for j < count, else 0

Sharding: data-parallel over batch. Each core takes 4 batches (16384 sites),
compacts its active rows to the front of its local [16384, 256] output and
reports its site mask. The host concatenates the 8 compacted segments (batch
blocks are contiguous in global site order, so this preserves the reference
row order) and zero-pads the tail.

Per-core device pipeline (per batch b of 4):
  - load x[b] as two [128, 4096] SBUF tiles (channel halves)
  - site abs-sums via ACT abs + PE ones-matmul (partition reduce) -> mask
  - exclusive prefix of mask: DVE free-dim scan over [32 chunks, 128 sites]
    + strict-upper-triangular matmul over chunk totals + scalar carry chain
  - dest row for site i: excl_i if active else 16383 - (i - excl_i)
    (a permutation of [0, 16384) -> every output row written exactly once,
     collision-free; inactive rows are multiplied by 0 so they deposit zeros)
  - transpose d to the wrapped int16 index layout dma_scatter_add expects
    (token i reads its index from [i % 16, i // 16]) via a DRAM roundtrip
  - PE-transpose each 128-site tile to [site, channel], multiply by the mask,
    and dma_scatter_add 2048 rows per call onto the zero-initialized output.
"""

import sys

sys.path.insert(0, "/opt/trn_rl_repo")

import numpy as np

_CACHE = {}

B_FULL = 32
C = 256
H = 64
W = 64
S = H * W                  # 4096 spatial sites per batch
N_CORES = 8
B_CORE = B_FULL // N_CORES  # 4 batches per core
N_LOC = B_CORE * S          # 16384 sites per core
P = 128
NCHUNK = S // P             # 32 chunks of 128 sites per batch
E = C                       # 256 elements per output row
TOK_PER_CALL = 2048         # dma_scatter_add rows per call
GROUPS = S // TOK_PER_CALL  # 2 calls per batch


def _build(loop_reps=None):
    """Build the per-core kernel. loop_reps wraps the whole body in an
    on-device For_i loop (timing only — output accumulates garbage)."""
    import contextlib

    import concourse.bacc as bacc
    import concourse.bass as bass
    import concourse.mybir as mybir
    from concourse.masks import make_identity, make_upper_triangular
    from concourse.tile import TileContext

    f32 = mybir.dt.float32
    i32 = mybir.dt.int32
    i16 = mybir.dt.int16

    nc = bacc.Bacc("TRN2", target_bir_lowering=False)
    x = nc.dram_tensor("x", [B_CORE, C, S], f32, kind="ExternalInput")
    out = nc.dram_tensor("out", [N_LOC, E], f32, kind="ExternalOutput")
    maskout = nc.dram_tensor("mask", [P, P], f32, kind="ExternalOutput")

    with TileContext(nc) as tc:
        with (
            tc.tile_pool(name="const", bufs=1) as cpool,
            tc.tile_pool(name="xin", bufs=2) as xpool,
            tc.tile_pool(name="xa", bufs=4) as xapool,
            tc.tile_pool(name="small", bufs=2) as spool,
            tc.tile_pool(name="fst", bufs=2) as fpool,
            tc.tile_pool(name="mps", bufs=2, space="PSUM") as mpspool,
            tc.tile_pool(name="sps", bufs=2, space="PSUM") as spspool,
            tc.tile_pool(name="fps", bufs=3, space="PSUM") as fpspool,
            tc.tile_pool(name="dscr", bufs=2, space="DRAM") as dpool,
        ):
            ident = cpool.tile([P, P], f32)
            make_identity(nc, ident[:])
            lsu = cpool.tile([NCHUNK, NCHUNK], f32)
            make_upper_triangular(nc, lsu[:], val=1.0, diag=False)
            ones_col = cpool.tile([P, 1], f32)
            nc.gpsimd.memset(ones_col[:], 1.0)
            ones_row32 = cpool.tile([1, NCHUNK], f32)
            nc.gpsimd.memset(ones_row32[:], 1.0)
            ones_col32 = cpool.tile([NCHUNK, 1], f32)
            nc.gpsimd.memset(ones_col32[:], 1.0)
            zeros32 = cpool.tile([NCHUNK, P], f32)
            nc.gpsimd.memset(zeros32[:], 0.0)
            vi = cpool.tile([NCHUNK, P], i32)
            nc.gpsimd.iota(vi[:], pattern=[[1, P]], base=0, channel_multiplier=P)
            vf = cpool.tile([NCHUNK, P], f32)
            nc.vector.tensor_copy(out=vf[:], in_=vi[:])
            # idxs_full[fl, i16col]: wrapped dest indices for all 16384 tokens
            idxs_full = cpool.tile([P, N_LOC // 16], i16)

            loop_cm = (
                tc.For_i(0, loop_reps, 1) if loop_reps else contextlib.nullcontext()
            )
            with loop_cm:
              carry_prev = None
              for b in range(B_CORE):
                xt0 = xpool.tile([P, S], f32, tag="x0")
                xt1 = xpool.tile([P, S], f32, tag="x1")
                nc.sync.dma_start(out=xt0[:], in_=x[b, 0:P, :])
                nc.sync.dma_start(out=xt1[:], in_=x[b, P : 2 * P, :])

                # --- site abs-sums -> Srow [1, 4096] ---
                srow = spool.tile([1, S], f32, tag="srow")
                for j in range(S // 512):
                    sl = slice(j * 512, (j + 1) * 512)
                    mps = mpspool.tile([1, 512], f32, tag="mps")
                    xa0 = xapool.tile([P, 512], f32, tag="xa")
                    nc.scalar.activation(
                        out=xa0[:], in_=xt0[:, sl], func=mybir.ActivationFunctionType.Abs
                    )
                    nc.tensor.matmul(
                        mps[:], lhsT=ones_col[:], rhs=xa0[:], start=True, stop=False
                    )
                    xa1 = xapool.tile([P, 512], f32, tag="xa")
                    nc.scalar.activation(
                        out=xa1[:], in_=xt1[:, sl], func=mybir.ActivationFunctionType.Abs
                    )
                    nc.tensor.matmul(
                        mps[:], lhsT=ones_col[:], rhs=xa1[:], start=False, stop=True
                    )
                    nc.vector.tensor_copy(out=srow[:, sl], in_=mps[:])

                # --- reshape [1, 4096] -> [32, 128] (raveled sbuf->sbuf DMA) ---
                s2 = spool.tile([NCHUNK, P], f32, tag="s2")
                nc.sync.dma_start(out=s2[:], in_=srow[:])

                a2 = spool.tile([NCHUNK, P], f32, tag="a2")
                nc.vector.tensor_scalar(
                    out=a2[:], in0=s2[:], scalar1=0.0, scalar2=None,
                    op0=mybir.AluOpType.is_gt,
                )
                nc.sync.dma_start(
                    out=maskout[b * NCHUNK : (b + 1) * NCHUNK, :], in_=a2[:]
                )

                # --- inclusive scan along sites within each chunk ---
                incl = spool.tile([NCHUNK, P], f32, tag="incl")
                nc.vector.tensor_tensor_scan(
                    out=incl[:], data0=a2[:], data1=zeros32[:], initial=0.0,
                    op0=mybir.AluOpType.add, op1=mybir.AluOpType.add,
                )

                # --- chunk-exclusive base: E[p] = sum_{q<p} T[q] (+ carry) ---
                eps = spspool.tile([NCHUNK, 1], f32, tag="sps")
                nc.tensor.matmul(
                    eps[:], lhsT=lsu[:], rhs=incl[:, P - 1 : P],
                    start=True, stop=(b == 0),
                )
                if b > 0:
                    nc.tensor.matmul(
                        eps[:], lhsT=ones_row32[:], rhs=carry_prev[:],
                        start=False, stop=True,
                    )
                esb = spool.tile([NCHUNK, 1], f32, tag="esb")
                nc.vector.tensor_copy(out=esb[:], in_=eps[:])

                # --- carry update: carry_b = carry_{b-1} + sum(T) ---
                tsum = spspool.tile([1, 1], f32, tag="sps")
                nc.tensor.matmul(
                    tsum[:], lhsT=incl[:, P - 1 : P], rhs=ones_col32[:],
                    start=True, stop=True,
                )
                carry = spool.tile([1, 1], f32, tag="carry")
                if b == 0:
                    nc.vector.tensor_copy(out=carry[:], in_=tsum[:])
                else:
                    nc.vector.tensor_tensor(
                        out=carry[:], in0=carry_prev[:], in1=tsum[0:1, 0:1],
                        op=mybir.AluOpType.add,
                    )
                carry_prev = carry

                # --- dest index d = excl + (1 - a) * (16383 - i) ---
                excl = spool.tile([NCHUNK, P], f32, tag="excl")
                nc.vector.tensor_tensor(
                    out=excl[:], in0=incl[:], in1=a2[:], op=mybir.AluOpType.subtract
                )
                nc.vector.tensor_tensor(
                    out=excl[:], in0=excl[:],
                    in1=esb[:, 0:1].to_broadcast([NCHUNK, P]),
                    op=mybir.AluOpType.add,
                )
                ri = spool.tile([NCHUNK, P], f32, tag="ri")
                nc.vector.tensor_scalar(
                    out=ri[:], in0=vf[:], scalar1=-1.0,
                    scalar2=float(N_LOC - 1 - b * S),
                    op0=mybir.AluOpType.mult, op1=mybir.AluOpType.add,
                )
                na = spool.tile([NCHUNK, P], f32, tag="na")
                nc.vector.tensor_scalar(
                    out=na[:], in0=a2[:], scalar1=-1.0, scalar2=1.0,
                    op0=mybir.AluOpType.mult, op1=mybir.AluOpType.add,
                )
                nc.vector.tensor_tensor(
                    out=na[:], in0=na[:], in1=ri[:], op=mybir.AluOpType.mult
                )
                df = spool.tile([NCHUNK, P], f32, tag="df")
                nc.vector.tensor_tensor(
                    out=df[:], in0=excl[:], in1=na[:], op=mybir.AluOpType.add
                )

                # --- transpose d and the mask to [site-in-chunk, chunk] ---
                dtps = spspool.tile([P, NCHUNK], f32, tag="sps")
                nc.tensor.transpose(
                    out=dtps[:], in_=df[:], identity=ident[0:NCHUNK, 0:NCHUNK]
                )
                dt16 = spool.tile([P, NCHUNK], i16, tag="dt16")
                nc.vector.tensor_copy(out=dt16[:], in_=dtps[:])

                amps = spspool.tile([P, NCHUNK], f32, tag="sps")
                nc.tensor.transpose(
                    out=amps[:], in_=a2[:], identity=ident[0:NCHUNK, 0:NCHUNK]
                )
                amt = spool.tile([P, NCHUNK], f32, tag="amt")
                nc.scalar.activation(
                    out=amt[:], in_=amps[:], func=mybir.ActivationFunctionType.Copy
                )

                # --- dt16 [128=(16fh+fl), 32=p'] -> idxs_full[fl, 256b+8p'+fh],
                #     replicated over the 8 groups of 16 partitions ---
                iscr = dpool.tile([16, 256], i16, tag="iscr")
                # write order (fh, fl, p') -> dram addr fl*256 + 8p' + fh
                wap = bass.AP(iscr[:].tensor, iscr[:].offset, [[1, 8], [256, 16], [8, 32]])
                nc.sync.dma_start(out=wap, in_=dt16[:])
                # read back (rep, fl, col) with the rep dim 0-strided
                rap = bass.AP(iscr[:].tensor, iscr[:].offset, [[0, 8], [256, 16], [1, 256]])
                nc.sync.dma_start(
                    out=idxs_full[:, b * 256 : (b + 1) * 256], in_=rap
                )

                # --- stage masked transposed rows and scatter ---
                for g in range(GROUPS):
                    fst = fpool.tile([P, (TOK_PER_CALL // P) * E], f32, tag="fst")
                    for tt in range(TOK_PER_CALL // P):
                        chunk = g * (TOK_PER_CALL // P) + tt
                        sl = slice(chunk * P, (chunk + 1) * P)
                        fps = fpspool.tile([P, E], f32, tag="fps")
                        nc.tensor.transpose(
                            out=fps[:, 0:P], in_=xt0[:, sl], identity=ident[:]
                        )
                        nc.tensor.transpose(
                            out=fps[:, P : 2 * P], in_=xt1[:, sl], identity=ident[:]
                        )
                        nc.vector.tensor_tensor(
                            out=fst[:, tt * E : (tt + 1) * E],
                            in0=fps[:],
                            in1=amt[:, chunk : chunk + 1].to_broadcast([P, E]),
                            op=mybir.AluOpType.mult,
                        )
                    col0 = (b * S + g * TOK_PER_CALL) // 16
                    nc.gpsimd.dma_scatter_add(
                        out[:],
                        fst[:].rearrange("p (s e) -> p s e", e=E),
                        idxs_full[:, col0 : col0 + TOK_PER_CALL // 16],
                        TOK_PER_CALL,
                        TOK_PER_CALL,
                        E,
                    )

    nc.compile()
    return nc


def _get_nc():
    if "nc" not in _CACHE:
        _CACHE["nc"] = _build()
    return _CACHE["nc"]


def kernel(x: np.ndarray) -> np.ndarray:
    from concourse.bass_utils import run_bass_kernel_spmd

    nc = _get_nc()
    x = np.ascontiguousarray(x, dtype=np.float32)
    in_maps = [
        {"x": np.ascontiguousarray(x[d * B_CORE : (d + 1) * B_CORE].reshape(B_CORE, C, S))}
        for d in range(N_CORES)
    ]
    res = run_bass_kernel_spmd(nc, in_maps, core_ids=list(range(N_CORES)))
    final = np.zeros((B_FULL * S, E), dtype=np.float32)
    off = 0
    for d in range(N_CORES):
        r = res.results[d]
        cnt = int(round(float(r["mask"].sum())))
        if cnt:
            final[off : off + cnt] = r["out"][:cnt]
        off += cnt
    return final


# revision 4
# speedup vs baseline: 1.2519x; 1.2519x over previous
"""DenseToSparse kernel for Trainium2 (8 NeuronCores, batch-parallel).

Reference computation (per full input x [32, 256, 64, 64] fp32):
  feats = x.transpose(0,2,3,1).reshape(-1, 256)       # [131072, 256]
  active = |feats|.sum(axis=1) > 0                     # site mask
  out[j] = feats[sorted_active_sites[j]] for j < count, else 0

Sharding: data-parallel over batch. Each core takes 4 batches (16384 sites),
compacts its active rows to the front of its local [16384, 256] output and
reports its site mask. The host concatenates the 8 compacted segments (batch
blocks are contiguous in global site order, so this preserves the reference
row order) and zero-pads the tail.

Per-core device pipeline (per batch b of 4):
  - load x[b] as two [128, 4096] SBUF tiles (channel halves)
  - site abs-sums via ACT abs + PE ones-matmul (partition reduce) -> mask
  - exclusive prefix of mask: DVE free-dim scan over [32 chunks, 128 sites]
    + strict-upper-triangular matmul over chunk totals + scalar carry chain
  - dest row for site i: excl_i if active else 16383 - (i - excl_i)
    (a permutation of [0, 16384) -> every output row written exactly once,
     collision-free; inactive rows are multiplied by 0 so they deposit zeros)
  - transpose d to the wrapped int16 index layout dma_scatter_add expects
    (token i reads its index from [i % 16, i // 16]) via a DRAM roundtrip
  - PE-transpose each 128-site tile to [site, channel], multiply by the mask,
    and dma_scatter_add 2048 rows per call onto the zero-initialized output.
"""

import sys

sys.path.insert(0, "/opt/trn_rl_repo")

import numpy as np

_CACHE = {}

B_FULL = 32
C = 256
H = 64
W = 64
S = H * W                  # 4096 spatial sites per batch
N_CORES = 8
B_CORE = B_FULL // N_CORES  # 4 batches per core
N_LOC = B_CORE * S          # 16384 sites per core
P = 128
NCHUNK = S // P             # 32 chunks of 128 sites per batch
E = C                       # 256 elements per output row
TOK_PER_CALL = 2048         # dma_scatter_add rows per call
GROUPS = S // TOK_PER_CALL  # 2 calls per batch


def _build(loop_reps=None):
    """Build the per-core kernel. loop_reps wraps the whole body in an
    on-device For_i loop (timing only — output accumulates garbage)."""
    import contextlib

    import concourse.bacc as bacc
    import concourse.bass as bass
    import concourse.mybir as mybir
    from concourse.masks import make_identity, make_upper_triangular
    from concourse.tile import TileContext

    f32 = mybir.dt.float32
    i32 = mybir.dt.int32
    i16 = mybir.dt.int16

    nc = bacc.Bacc("TRN2", target_bir_lowering=False)
    x = nc.dram_tensor("x", [B_CORE, C, S], f32, kind="ExternalInput")
    out = nc.dram_tensor("out", [N_LOC, E], f32, kind="ExternalOutput")
    maskout = nc.dram_tensor("mask", [P, P], f32, kind="ExternalOutput")

    with TileContext(nc) as tc:
        with (
            tc.tile_pool(name="const", bufs=1) as cpool,
            tc.tile_pool(name="xin", bufs=2) as xpool,
            tc.tile_pool(name="xa", bufs=4) as xapool,
            tc.tile_pool(name="small", bufs=2) as spool,
            tc.tile_pool(name="fst", bufs=2) as fpool,
            tc.tile_pool(name="mps", bufs=2, space="PSUM") as mpspool,
            tc.tile_pool(name="sps", bufs=2, space="PSUM") as spspool,
            tc.tile_pool(name="fps", bufs=3, space="PSUM") as fpspool,
            tc.tile_pool(name="dscr", bufs=2, space="DRAM") as dpool,
        ):
            ident = cpool.tile([P, P], f32)
            make_identity(nc, ident[:])
            lsu = cpool.tile([NCHUNK, NCHUNK], f32)
            make_upper_triangular(nc, lsu[:], val=1.0, diag=False)
            ones_col = cpool.tile([P, 1], f32)
            nc.gpsimd.memset(ones_col[:], 1.0)
            ones_row32 = cpool.tile([1, NCHUNK], f32)
            nc.gpsimd.memset(ones_row32[:], 1.0)
            ones_col32 = cpool.tile([NCHUNK, 1], f32)
            nc.gpsimd.memset(ones_col32[:], 1.0)
            zeros32 = cpool.tile([NCHUNK, P], f32)
            nc.gpsimd.memset(zeros32[:], 0.0)
            vi = cpool.tile([NCHUNK, P], i32)
            nc.gpsimd.iota(vi[:], pattern=[[1, P]], base=0, channel_multiplier=P)
            vf = cpool.tile([NCHUNK, P], f32)
            nc.vector.tensor_copy(out=vf[:], in_=vi[:])
            # idxs_full[fl, i16col]: wrapped dest indices for all 16384 tokens
            idxs_full = cpool.tile([P, N_LOC // 16], i16)

            loop_cm = (
                tc.For_i(0, loop_reps, 1) if loop_reps else contextlib.nullcontext()
            )
            with loop_cm:
              carry_prev = None
              for b in range(B_CORE):
                xt0 = xpool.tile([P, S], f32, tag="x0")
                xt1 = xpool.tile([P, S], f32, tag="x1")
                nc.sync.dma_start(out=xt0[:], in_=x[b, 0:P, :])
                nc.sync.dma_start(out=xt1[:], in_=x[b, P : 2 * P, :])

                # --- site abs-sums -> Srow [1, 4096] ---
                srow = spool.tile([1, S], f32, tag="srow")
                for j in range(S // 512):
                    sl = slice(j * 512, (j + 1) * 512)
                    mps = mpspool.tile([1, 512], f32, tag="mps")
                    xa0 = xapool.tile([P, 512], f32, tag="xa")
                    nc.scalar.activation(
                        out=xa0[:], in_=xt0[:, sl], func=mybir.ActivationFunctionType.Abs
                    )
                    nc.tensor.matmul(
                        mps[:], lhsT=ones_col[:], rhs=xa0[:], start=True, stop=False
                    )
                    xa1 = xapool.tile([P, 512], f32, tag="xa")
                    nc.scalar.activation(
                        out=xa1[:], in_=xt1[:, sl], func=mybir.ActivationFunctionType.Abs
                    )
                    nc.tensor.matmul(
                        mps[:], lhsT=ones_col[:], rhs=xa1[:], start=False, stop=True
                    )
                    nc.vector.tensor_copy(out=srow[:, sl], in_=mps[:])

                # --- reshape [1, 4096] -> [32, 128] (raveled sbuf->sbuf DMA) ---
                s2 = spool.tile([NCHUNK, P], f32, tag="s2")
                nc.sync.dma_start(out=s2[:], in_=srow[:])

                a2 = spool.tile([NCHUNK, P], f32, tag="a2")
                nc.vector.tensor_scalar(
                    out=a2[:], in0=s2[:], scalar1=0.0, scalar2=None,
                    op0=mybir.AluOpType.is_gt,
                )
                nc.sync.dma_start(
                    out=maskout[b * NCHUNK : (b + 1) * NCHUNK, :], in_=a2[:]
                )

                # --- inclusive scan along sites within each chunk ---
                incl = spool.tile([NCHUNK, P], f32, tag="incl")
                nc.vector.tensor_tensor_scan(
                    out=incl[:], data0=a2[:], data1=zeros32[:], initial=0.0,
                    op0=mybir.AluOpType.add, op1=mybir.AluOpType.add,
                )

                # --- chunk-exclusive base: E[p] = sum_{q<p} T[q] (+ carry) ---
                eps = spspool.tile([NCHUNK, 1], f32, tag="sps")
                nc.tensor.matmul(
                    eps[:], lhsT=lsu[:], rhs=incl[:, P - 1 : P],
                    start=True, stop=(b == 0),
                )
                if b > 0:
                    nc.tensor.matmul(
                        eps[:], lhsT=ones_row32[:], rhs=carry_prev[:],
                        start=False, stop=True,
                    )
                esb = spool.tile([NCHUNK, 1], f32, tag="esb")
                nc.vector.tensor_copy(out=esb[:], in_=eps[:])

                # --- carry update: carry_b = carry_{b-1} + sum(T) ---
                tsum = spspool.tile([1, 1], f32, tag="sps")
                nc.tensor.matmul(
                    tsum[:], lhsT=incl[:, P - 1 : P], rhs=ones_col32[:],
                    start=True, stop=True,
                )
                carry = spool.tile([1, 1], f32, tag="carry")
                if b == 0:
                    nc.vector.tensor_copy(out=carry[:], in_=tsum[:])
                else:
                    nc.vector.tensor_tensor(
                        out=carry[:], in0=carry_prev[:], in1=tsum[0:1, 0:1],
                        op=mybir.AluOpType.add,
                    )
                carry_prev = carry

                # --- dest index d = excl + (1 - a) * (16383 - i) ---
                excl = spool.tile([NCHUNK, P], f32, tag="excl")
                nc.vector.tensor_tensor(
                    out=excl[:], in0=incl[:], in1=a2[:], op=mybir.AluOpType.subtract
                )
                nc.vector.tensor_tensor(
                    out=excl[:], in0=excl[:],
                    in1=esb[:, 0:1].to_broadcast([NCHUNK, P]),
                    op=mybir.AluOpType.add,
                )
                ri = spool.tile([NCHUNK, P], f32, tag="ri")
                nc.vector.tensor_scalar(
                    out=ri[:], in0=vf[:], scalar1=-1.0,
                    scalar2=float(N_LOC - 1 - b * S),
                    op0=mybir.AluOpType.mult, op1=mybir.AluOpType.add,
                )
                na = spool.tile([NCHUNK, P], f32, tag="na")
                nc.vector.tensor_scalar(
                    out=na[:], in0=a2[:], scalar1=-1.0, scalar2=1.0,
                    op0=mybir.AluOpType.mult, op1=mybir.AluOpType.add,
                )
                nc.vector.tensor_tensor(
                    out=na[:], in0=na[:], in1=ri[:], op=mybir.AluOpType.mult
                )
                df = spool.tile([NCHUNK, P], f32, tag="df")
                nc.vector.tensor_tensor(
                    out=df[:], in0=excl[:], in1=na[:], op=mybir.AluOpType.add
                )

                # --- transpose d and the mask to [site-in-chunk, chunk] ---
                dtps = spspool.tile([P, NCHUNK], f32, tag="sps")
                nc.tensor.transpose(
                    out=dtps[:], in_=df[:], identity=ident[0:NCHUNK, 0:NCHUNK]
                )
                dt16 = spool.tile([P, NCHUNK], i16, tag="dt16")
                nc.vector.tensor_copy(out=dt16[:], in_=dtps[:])

                amps = spspool.tile([P, NCHUNK], f32, tag="sps")
                nc.tensor.transpose(
                    out=amps[:], in_=a2[:], identity=ident[0:NCHUNK, 0:NCHUNK]
                )
                amt = spool.tile([P, NCHUNK], f32, tag="amt")
                nc.scalar.activation(
                    out=amt[:], in_=amps[:], func=mybir.ActivationFunctionType.Copy
                )

                # --- dt16 [128=(16fh+fl), 32=p'] -> idxs_full[fl, 256b+8p'+fh],
                #     replicated over the 8 groups of 16 partitions ---
                iscr = dpool.tile([16, 256], i16, tag="iscr")
                # write order (fh, fl, p') -> dram addr fl*256 + 8p' + fh
                wap = bass.AP(iscr[:].tensor, iscr[:].offset, [[1, 8], [256, 16], [8, 32]])
                nc.sync.dma_start(out=wap, in_=dt16[:])
                # read back (rep, fl, col) with the rep dim 0-strided
                rap = bass.AP(iscr[:].tensor, iscr[:].offset, [[0, 8], [256, 16], [1, 256]])
                nc.sync.dma_start(
                    out=idxs_full[:, b * 256 : (b + 1) * 256], in_=rap
                )

                # --- stage masked transposed rows and scatter ---
                for g in range(GROUPS):
                    fst = fpool.tile([P, (TOK_PER_CALL // P) * E], f32, tag="fst")
                    for tt in range(TOK_PER_CALL // P):
                        chunk = g * (TOK_PER_CALL // P) + tt
                        sl = slice(chunk * P, (chunk + 1) * P)
                        fps = fpspool.tile([P, E], f32, tag="fps")
                        nc.tensor.transpose(
                            out=fps[:, 0:P], in_=xt0[:, sl], identity=ident[:]
                        )
                        nc.tensor.transpose(
                            out=fps[:, P : 2 * P], in_=xt1[:, sl], identity=ident[:]
                        )
                        nc.vector.tensor_tensor(
                            out=fst[:, tt * E : (tt + 1) * E],
                            in0=fps[:],
                            in1=amt[:, chunk : chunk + 1].to_broadcast([P, E]),
                            op=mybir.AluOpType.mult,
                        )
                    col0 = (b * S + g * TOK_PER_CALL) // 16
                    nc.gpsimd.dma_scatter_add(
                        out[:],
                        fst[:].rearrange("p (s e) -> p s e", e=E),
                        idxs_full[:, col0 : col0 + TOK_PER_CALL // 16],
                        TOK_PER_CALL,
                        TOK_PER_CALL,
                        E,
                        single_packet=False,
                    )

    nc.compile()
    return nc


def _get_nc():
    if "nc" not in _CACHE:
        _CACHE["nc"] = _build()
    return _CACHE["nc"]


def kernel(x: np.ndarray) -> np.ndarray:
    from concourse.bass_utils import run_bass_kernel_spmd

    nc = _get_nc()
    x = np.ascontiguousarray(x, dtype=np.float32)
    in_maps = [
        {"x": np.ascontiguousarray(x[d * B_CORE : (d + 1) * B_CORE].reshape(B_CORE, C, S))}
        for d in range(N_CORES)
    ]
    res = run_bass_kernel_spmd(nc, in_maps, core_ids=list(range(N_CORES)))
    final = np.zeros((B_FULL * S, E), dtype=np.float32)
    off = 0
    for d in range(N_CORES):
        r = res.results[d]
        cnt = int(round(float(r["mask"].sum())))
        if cnt:
            final[off : off + cnt] = r["out"][:cnt]
        off += cnt
    return final
